# revision 42
# baseline (speedup 1.0000x reference)
"""Trainium2 Bass kernel for MC-sampled cross-entropy-with-variance loss.

Computes mean over (s, b, h, w) of
    nll = logsumexp_c(mean + exp(0.5*log_var)*eps[s]) - logit[label]
distributed over 8 NeuronCores by sharding the H*W pixel axis.

v2 architecture (DVE was the v1 bottleneck at 86% busy):
- Elementwise work batched into multi-sample granules [114, nsl*2048]:
  one TT mul (eps*std, std broadcast via stride-0 AP over samples) +
  one in-place TT add (+mean) per granule -> exactly 2 DVE passes per
  eps element at 2x bf16 mode, ~100 large DVE ops total.
- exp runs on ACT in-place over the whole granule.
- Sum over classes on the PE with a sample-invariant chunk-selector
  [114, g] (rows = chunk id); per-sample outputs land at PSUM
  partition bases 32*sl (column tiling via tile_position), drained per
  granule with one grouped-partition ACT copy into a collect buffer
  for a deferred per-image ln pass.
- The label side needs sum_s logits[label]: the PE accumulates
  acc = sum_s t2 (identity weights) into a [114, 2048] PSUM tile per
  region; one tensor_tensor_reduce(mask * acc) per region chains into
  a [114,1] fp32 accumulator. This removes the per-sample eps
  accumulation pass that used to run on the DVE.
"""

import numpy as np
import ml_dtypes

import concourse.bass as bass
import concourse.bacc as bacc
import concourse.mybir as mybir
from concourse import tile
from concourse.bass_interp import get_hw_module
from concourse.bass_utils import run_bass_kernel_spmd
from concourse.mybir import AluOpType as Alu
from concourse.mybir import ActivationFunctionType as Act

# ---------------------------------------------------------------- sizes
S, B, C, H, W = 10, 4, 19, 512, 512
HW = H * W
NCORES = 8
SLAB = HW // NCORES          # pixels per (core, b) = 32768
F = 2048                     # free-dim pixels per chunk
G_FULL = 6                   # chunks packed per full region (6*19=114 parts)
REGIONS = [
    (G_FULL, 0 * F),         # (num chunks, pixel offset)
    (G_FULL, 6 * F),
    (4, 12 * F),
]
NREG = len(REGIONS)
GRANULES = [(0, 3), (3, 3), (6, 3), (9, 1)]   # (start sample, num samples)
# PSUM column bases 32*sl: only {0,32,64} are legal (quadrant 3 is buggy)
MM_N = 512                   # matmul free-dim (PSUM bank limit)
F32 = mybir.dt.float32
BF16 = mybir.dt.bfloat16


def _region_ap(handle, base_off, poff, g, row_stride):
    """DRAM AP for a [19*g, F] tile: partitions iterate (chunk j, class c)
    chunk-outer. row_stride==0 broadcasts pixels across class rows."""
    return bass.AP(
        tensor=handle,
        offset=base_off + poff,
        ap=[[F, g], [row_stride, C], [1, F]],
    )


def _granule_ap(handle, b, s0, nsl, poff, g):
    """DRAM AP for a [19*g, nsl*F] eps granule: free iterates (sample,
    pixel)."""
    return bass.AP(
        tensor=handle,
        offset=(s0 * B + b) * C * SLAB + poff,
        ap=[[F, g], [SLAB, C], [B * C * SLAB, nsl], [1, F]],
    )


def build_program():
    nc = bacc.Bacc("TRN2", target_bir_lowering=False, debug=False,
                   num_devices=NCORES)

    eps_h = nc.dram_tensor("eps_s", [B, 16 * C, S, F], BF16, kind="ExternalInput")
    mean_h = nc.dram_tensor("mean_s", [B, C, SLAB], BF16, kind="ExternalInput")
    lv_h = nc.dram_tensor("lv_s", [B, C, SLAB], BF16, kind="ExternalInput")
    lab_h = nc.dram_tensor("lab_s", [B, SLAB], BF16, kind="ExternalInput")
    cvec6_h = nc.dram_tensor("cvec6", [114, 1], F32, kind="ExternalInput")
    cvec4_h = nc.dram_tensor("cvec4", [76, 1], F32, kind="ExternalInput")
    sel6_h = nc.dram_tensor("sel6", [114, 6], BF16, kind="ExternalInput")
    sel4_h = nc.dram_tensor("sel4", [76, 4], BF16, kind="ExternalInput")
    id6_h = nc.dram_tensor("id6", [114, 114], BF16, kind="ExternalInput")
    id4_h = nc.dram_tensor("id4", [76, 76], BF16, kind="ExternalInput")
    lse_h = nc.dram_tensor("lse_out", [128, 1], F32, kind="ExternalOutput")
    lse4_h = nc.dram_tensor("lse4_out", [128, 1], F32, kind="ExternalOutput")
    lab_o_h = nc.dram_tensor("lab_out", [114, 1], F32, kind="ExternalOutput")

    with tile.TileContext(nc) as tc:
        with (
            tc.tile_pool(name="consts", bufs=1) as consts,
            tc.tile_pool(name="region", bufs=2) as region_pool,
            tc.tile_pool(name="epsp", bufs=3) as eps_pool,
            tc.tile_pool(name="work", bufs=3) as work_pool,
            tc.tile_pool(name="coll", bufs=2) as coll_pool,
            tc.tile_pool(name="accp", bufs=1) as acc_pool,
            tc.tile_pool(name="ps_acc", bufs=1, space="PSUM") as ps_acc_pool,
            tc.tile_pool(name="ps_se", bufs=2, space="PSUM") as ps_se_pool,
        ):
            cvec6_sb = consts.tile([114, 1], F32)
            nc.sync.dma_start(out=cvec6_sb, in_=cvec6_h.ap())
            cvec4_sb = consts.tile([76, 1], F32)
            nc.sync.dma_start(out=cvec4_sb, in_=cvec4_h.ap())
            sel6_sb = consts.tile([114, 6], BF16)
            nc.sync.dma_start(out=sel6_sb, in_=sel6_h.ap())
            sel4_sb = consts.tile([76, 4], BF16)
            nc.sync.dma_start(out=sel4_sb, in_=sel4_h.ap())
            id6_sb = consts.tile([114, 114], BF16)
            nc.sync.dma_start(out=id6_sb, in_=id6_h.ap())
            id4_sb = consts.tile([76, 76], BF16)
            nc.sync.dma_start(out=id4_sb, in_=id4_h.ap())

            acc_lse6 = acc_pool.tile([128, 1], F32)
            nc.vector.memset(acc_lse6, 0.0)
            acc_lse4 = acc_pool.tile([128, 1], F32)
            nc.vector.memset(acc_lse4, 0.0)
            acc_lab = acc_pool.tile([114, 1], F32)
            nc.vector.memset(acc_lab, 0.0)
            junk = acc_pool.tile([114, F], BF16)

            for b in range(B):
                for r, (g, poff) in enumerate(REGIONS):
                    p_ = g * C          # active partitions (114 or 76)
                    sel_sb = sel6_sb if g == G_FULL else sel4_sb
                    id_sb = id6_sb if g == G_FULL else id4_sb
                    cvec_sb = cvec6_sb if g == G_FULL else cvec4_sb

                    mean_sb = region_pool.tile([114, F], BF16, tag="mean")
                    nc.sync.dma_start(
                        out=mean_sb[:p_, :],
                        in_=_region_ap(mean_h, b * C * SLAB, poff, g, SLAB),
                    )
                    lv_t = region_pool.tile([114, F], BF16, tag="lv")
                    nc.sync.dma_start(
                        out=lv_t[:p_, :],
                        in_=_region_ap(lv_h, b * C * SLAB, poff, g, SLAB),
                    )
                    std_sb = region_pool.tile([114, F], BF16, tag="std")
                    nc.scalar.activation(std_sb[:p_], lv_t[:p_], Act.Exp,
                                         scale=0.5)
                    lab_t = region_pool.tile([114, F], BF16, tag="lab")
                    nc.sync.dma_start(
                        out=lab_t[:p_, :],
                        in_=_region_ap(lab_h, b * SLAB, poff, g, 0),
                    )
                    mask_t = region_pool.tile([114, F], BF16, tag="mask")
                    nc.vector.tensor_scalar(
                        mask_t[:p_], lab_t[:p_], cvec_sb[:p_], None,
                        Alu.is_equal,
                    )

                    accw = region_pool.tile([114, 3 * F], BF16, tag="accw")
                    pending = []
                    for q, (s0, nsl) in enumerate(GRANULES):
                        et = eps_pool.tile([114, 3 * F], BF16, tag="et")
                        j0 = poff // F
                        nc.sync.dma_start(
                            out=et[:p_, : nsl * F],
                            in_=bass.AP(
                                tensor=eps_h,
                                offset=(b * 16 * C + j0 * C) * S * F
                                + s0 * F,
                                ap=[[S * F, p_], [1, nsl * F]],
                            ),
                        )
                        t2 = work_pool.tile([114, 3 * F], BF16, tag="t2")
                        et3 = et[:p_, : nsl * F].rearrange(
                            "p (s f) -> p s f", s=nsl)
                        t23 = t2[:p_, : nsl * F].rearrange(
                            "p (s f) -> p s f", s=nsl)
                        std_b = std_sb[:p_, :].unsqueeze(1).broadcast_to(
                            (p_, nsl, F))
                        mean_b = mean_sb[:p_, :].unsqueeze(1).broadcast_to(
                            (p_, nsl, F))
                        nc.vector.tensor_mul(t23, et3, std_b)
                        nc.vector.tensor_add(t23, t23, mean_b)

                        # DVE: accumulate sum_s t2 granule-wide (slots
                        # folded at region end)
                        if q == 0:
                            nc.vector.tensor_copy(accw[:p_, : nsl * F],
                                                  t2[:p_, : nsl * F])
                        else:
                            nc.vector.tensor_add(
                                accw[:p_, : nsl * F],
                                accw[:p_, : nsl * F],
                                t2[:p_, : nsl * F],
                            )

                        # ACT: exp into a separate tile
                        e1 = eps_pool.tile([114, 3 * F], BF16, tag="e1")
                        nc.scalar.activation(e1[:p_, : nsl * F],
                                             t2[:p_, : nsl * F], Act.Exp)

                        # PE: per-sample chunk sums of exp'd tile at
                        # column-tile positions 32*sl
                        se_ps = ps_se_pool.tile([128, F], F32, tag="seps")
                        for sl in range(nsl):
                            for n in range(F // MM_N):
                                nc.tensor.matmul(
                                    se_ps[32 * sl: 32 * sl + g,
                                          n * MM_N:(n + 1) * MM_N],
                                    sel_sb,
                                    e1[:p_, sl * F + n * MM_N:
                                       sl * F + (n + 1) * MM_N],
                                    start=True, stop=True,
                                    skip_group_check=True,
                                )
                        # defer ln to granule pairs (exp,exp,ln,ln)
                        # so the ACT exp/ln tables switch half as often
                        pending.append((se_ps, 32 * nsl))
                        if q % 2 == 1 or q == len(GRANULES) - 1:
                            acc_lse = (acc_lse6 if g == G_FULL
                                       else acc_lse4)
                            for se_p_t, pr in pending:
                                lnb = coll_pool.tile([128, F], BF16,
                                                     tag="lnb")
                                lse_p = acc_pool.tile([128, 1], F32,
                                                      tag="lsep", bufs=2)
                                nc.scalar.activation(
                                    lnb[:pr], se_p_t[:pr], Act.Ln,
                                    accum_out=lse_p[:pr])
                                nc.vector.tensor_add(
                                    acc_lse[:pr], acc_lse[:pr],
                                    lse_p[:pr])
                            pending = []

                    # fold the 3 sample-slots, then masked reduce
                    nc.vector.tensor_add(accw[:p_, :F], accw[:p_, :F],
                                         accw[:p_, F:2 * F])
                    nc.vector.tensor_add(accw[:p_, :F], accw[:p_, :F],
                                         accw[:p_, 2 * F:3 * F])
                    lab_p = region_pool.tile([114, 1], F32, tag="labp")
                    nc.vector.scalar_tensor_tensor(
                        junk[:p_, :], accw[:p_, :F], 1.0, mask_t[:p_],
                        Alu.mult, Alu.mult, accum_out=lab_p[:p_],
                    )
                    nc.vector.tensor_add(acc_lab[:p_], acc_lab[:p_],
                                         lab_p[:p_])

            nc.sync.dma_start(out=lse_h.ap(), in_=acc_lse6)
            nc.sync.dma_start(out=lse4_h.ap(), in_=acc_lse4)
            nc.sync.dma_start(out=lab_o_h.ap(), in_=acc_lab)

    nc.compile()
    nc.m = get_hw_module(nc.m)
    return nc


def _consts():
    # partition p = j * 19 + c  (chunk-outer, class-inner)
    cvec6 = (np.arange(114) % C).astype(np.float32).reshape(114, 1)
    cvec4 = (np.arange(76) % C).astype(np.float32).reshape(76, 1)
    sel6 = np.zeros((114, 6), dtype=ml_dtypes.bfloat16)
    sel4 = np.zeros((76, 4), dtype=ml_dtypes.bfloat16)
    for p in range(114):
        sel6[p, p // C] = 1.0
    for p in range(76):
        sel4[p, p // C] = 1.0
    id6 = np.eye(114, dtype=ml_dtypes.bfloat16)
    id4 = np.eye(76, dtype=ml_dtypes.bfloat16)
    return cvec6, cvec4, sel6, sel4, id6, id4


def kernel(mean, log_var, label, eps, _trace=False):
    mean = np.asarray(mean, dtype=np.float32).reshape(B, C, HW)
    log_var = np.asarray(log_var, dtype=np.float32).reshape(B, C, HW)
    label_f = np.asarray(label).reshape(B, HW).astype(ml_dtypes.bfloat16)
    eps_r = np.asarray(eps, dtype=np.float32).reshape(S, B, C, HW)

    cvec6, cvec4, sel6, sel4, id6, id4 = _consts()
    in_maps = []
    for c in range(NCORES):
        lo, hi = c * SLAB, (c + 1) * SLAB
        in_maps.append({
            "eps_s": np.ascontiguousarray(
                eps_r[:, :, :, lo:hi]
                .reshape(S, B, C, 16, F)
                .transpose(1, 3, 2, 0, 4)
                .reshape(B, 16 * C, S, F)
                .astype(ml_dtypes.bfloat16)),
            "mean_s": mean[:, :, lo:hi].astype(ml_dtypes.bfloat16),
            "lv_s": log_var[:, :, lo:hi].astype(ml_dtypes.bfloat16),
            "lab_s": np.ascontiguousarray(label_f[:, lo:hi]),
            "cvec6": cvec6,
            "cvec4": cvec4,
            "sel6": sel6,
            "sel4": sel4,
            "id6": id6,
            "id4": id4,
        })

    nc = build_program()
    res = run_bass_kernel_spmd(
        nc, in_maps, core_ids=list(range(NCORES)), trace=_trace
    )
    global last_results
    last_results = res

    idx6 = [32 * a + j for a in range(4) for j in range(6)]
    idx4 = [32 * a + j for a in range(4) for j in range(4)]
    total = np.float64(0.0)
    for c in range(NCORES):
        total += res.results[c]["lse_out"].astype(np.float64)[idx6].sum()
        total += res.results[c]["lse4_out"].astype(np.float64)[idx4].sum()
        total -= res.results[c]["lab_out"].astype(np.float64).sum()
    loss = total / float(S * B * HW)
    return np.float32(loss)
